# revision 1
# baseline (speedup 1.0000x reference)
"""DTNNStep graph-message-passing kernel for 8x Trainium2 NeuronCores.

Strategy: distance_membership_i is sorted, so pairs are sharded by
destination-atom range (6250 atoms per core -> contiguous pair range per
core). Each core processes its pairs in 128-atom "windows"; within a
window, pairs are padded to a fixed capacity (TPW tiles of 128) so the
instruction stream is identical across cores (SPMD). The segment sum is
a matmul with a one-hot selection matrix generated on-device from
host-precomputed window-relative indices. No collectives are needed:
each core owns a disjoint slice of the output.

The afh gather table lives in DRAM with two zero rows; the per-pair
gather afh[j] runs as two dma_gather ops per window (int16 indices are
signed, so j is split at 32256 with out-of-range slots pointing at a
zero row) summed on DVE. The matmul value path runs in bf16; segment
accumulation stays in f32 PSUM.
"""

import sys

for _p in ("/opt/trn_rl_repo",):
    if _p not in sys.path:
        sys.path.insert(0, _p)

import numpy as np
import ml_dtypes
import concourse.bass as bass
import concourse.bacc as bacc
import concourse.tile as tile
from concourse import mybir
from concourse.bass_utils import run_bass_kernel_spmd

F32 = mybir.dt.float32
BF16 = mybir.dt.bfloat16
I16 = mybir.dt.int16
NPBF = ml_dtypes.bfloat16

P = 128
N_ATOMS = 50000
N_PAIRS = 800000
N_EMB = 128
NCORES = 8
APC = N_ATOMS // NCORES            # atoms per core: 6250
NWIN = (APC + P - 1) // P          # windows per core: 49
APC_PAD = NWIN * P                 # 6272
TPW = 18                           # pair tiles per window
CAP = TPW * P                      # pair capacity per window: 2304
NTBL = 50176                       # table rows (50002 used, padded)
TBL_CH = NTBL // 512               # phase-A chunks: 98
SPLIT = 63 * 512                   # 32256: j >= SPLIT served by gather B
C16W = 5 * P                       # bf16 const pack width
C32W = 3 * P + 512                 # f32 const pack width (incl bcf bcast)


def build_nc():
    nc = bacc.Bacc()

    distT = nc.declare_dram_parameter("distT", [101, NWIN * CAP], BF16,
                                      isOutput=False)
    jidx = nc.declare_dram_parameter("jidx", [P, NWIN * TPW],
                                     mybir.dt.int32, isOutput=False)
    iprime = nc.declare_dram_parameter("iprime", [P, NWIN * TPW], F32,
                                       isOutput=False)
    afT = nc.declare_dram_parameter("afT", [P, NTBL], BF16, isOutput=False)
    af_own = nc.declare_dram_parameter("af_own", [APC_PAD, P], F32,
                                       isOutput=False)
    afT_own = nc.declare_dram_parameter("afT_own", [P, APC_PAD], BF16,
                                        isOutput=False)
    cp16_d = nc.declare_dram_parameter("cp16", [P, C16W], BF16, isOutput=False)
    cp32_d = nc.declare_dram_parameter("cp32", [P, C32W], F32, isOutput=False)
    out_d = nc.declare_dram_parameter("out", [APC_PAD, P], F32, isOutput=True)

    with tile.TileContext(nc) as tc:
        with (
            tc.tile_pool(name="dramtbl", bufs=1, space="DRAM") as tbl_pool,
            tc.tile_pool(name="consts", bufs=1) as cpool,
            tc.tile_pool(name="aft", bufs=4) as aft_pool,
            tc.tile_pool(name="afh", bufs=4) as afh_pool,
            tc.tile_pool(name="dist", bufs=3) as dist_pool,
            tc.tile_pool(name="gth", bufs=6) as gth_pool,
            tc.tile_pool(name="idx", bufs=3) as idx_pool,
            tc.tile_pool(name="fused", bufs=3) as fused_pool,
            tc.tile_pool(name="fusedT", bufs=3) as fusedT_pool,
            tc.tile_pool(name="msgs_sb", bufs=3) as msgs_sb_pool,
            tc.tile_pool(name="sgen", bufs=6) as s_pool,
            tc.tile_pool(name="flush", bufs=3) as fl_pool,
            tc.tile_pool(name="ps_dh", bufs=2, space="PSUM") as dh_ps,
            tc.tile_pool(name="ps_tp", bufs=2, space="PSUM") as tp_ps,
            tc.tile_pool(name="ps_msgs", bufs=2, space="PSUM") as msgs_ps,
            tc.tile_pool(name="ps_win", bufs=2, space="PSUM") as win_ps,
        ):
            table = tbl_pool.tile([NTBL + 2, P], F32)

            cpk = cpool.tile([P, C16W], BF16)
            nc.sync.dma_start(cpk[:], cp16_d[:])
            wcf = cpk[:, 0:P]
            wdfe = cpk[:101, P:2 * P]
            wfc = cpk[:, 2 * P:3 * P]
            iota = cpk[:, 3 * P:4 * P]
            ident = cpk[:, 4 * P:5 * P]
            cpk32 = cpool.tile([P, C32W], F32)
            nc.sync.dma_start(cpk32[:], cp32_d[:])
            bdf = cpk32[:, 0:1]
            bcf = cpk32[0:1, P:2 * P]
            ones = cpk32[0:1, 2 * P:3 * P]
            bcfb = cpk32[:, 3 * P:3 * P + 512]

            jall = cpool.tile([P, NWIN * TPW], mybir.dt.int32)
            nc.sync.dma_start(jall[:], jidx[:])
            iall = cpool.tile([P, NWIN * TPW], F32)
            nc.sync.dma_start(iall[:], iprime[:])

            # zero rows of the gather table (rows 0 and SPLIT+1)
            zrow = cpool.tile([1, P], F32)
            nc.gpsimd.memset(zrow[:], 0.0)
            nc.sync.dma_start(table[0:1, :], zrow[:])
            nc.sync.dma_start(table[SPLIT + 1:SPLIT + 2, :], zrow[:])

            # ---- phase A: afh table = af @ W_cf + b_cf (bf16, shifted) ----
            for ch in range(TBL_CH):
                a = aft_pool.tile([P, 512], BF16)
                nc.sync.dma_start(a[:], afT[:, ch * 512:(ch + 1) * 512])
                ps = dh_ps.tile([P, 512], F32, tag="dh")
                for s in range(4):
                    nc.tensor.matmul(ps[:, s * P:(s + 1) * P],
                                     lhsT=a[:, s * P:(s + 1) * P],
                                     rhs=wcf, start=True, stop=True)
                o = afh_pool.tile([P, 512], F32)
                nc.vector.tensor_tensor(o[:], ps[:], bcfb,
                                        op=mybir.AluOpType.add)
                r0 = ch * 512 + (1 if ch < 63 else 2)
                dst = table[r0:r0 + 512, :].rearrange("(s p) h -> p s h", p=P)
                nc.sync.dma_start(dst, o[:].rearrange("p (s h) -> p s h", h=P))

            tc.strict_bb_all_engine_barrier()

            # ---- phase B: main pair loop ----
            for w in range(NWIN):
                jt = jall[:, w * TPW:(w + 1) * TPW]
                it = iall[:, w * TPW:(w + 1) * TPW]
                dt = dist_pool.tile([101, CAP], BF16)
                nc.sync.dma_start(dt[:], distT[:, w * CAP:(w + 1) * CAP])

                gt = gth_pool.tile([P, TPW * P], F32, tag="gt")
                for k in range(TPW):
                    nc.gpsimd.indirect_dma_start(
                        out=gt[:, k * P:(k + 1) * P],
                        out_offset=None,
                        in_=table[:],
                        in_offset=bass.IndirectOffsetOnAxis(
                            ap=jt[:, k:k + 1], axis=0),
                    )

                win = win_ps.tile([P, P], F32)

                k = 0
                blk = 0
                while k < TPW:
                    nblk = min(4, TPW - k)
                    nb = nblk * P
                    dh = dh_ps.tile([P, 512], F32, tag="dh")
                    for s in range(nblk):
                        nc.tensor.matmul(
                            dh[:, s * P:(s + 1) * P],
                            lhsT=dt[:, (k + s) * P:(k + s + 1) * P],
                            rhs=wdfe, start=True, stop=True)
                    fused = fused_pool.tile([P, 512], BF16)
                    nc.vector.tensor_tensor(
                        fused[:, :nb], dh[:, :nb],
                        gt[:, k * P:k * P + nb], op=mybir.AluOpType.mult)
                    tp = tp_ps.tile([P, 512], BF16)
                    for s in range(nblk):
                        nc.tensor.transpose(
                            tp[:, s * P:(s + 1) * P],
                            fused[:, s * P:(s + 1) * P], ident)
                    fusedT = fusedT_pool.tile([P, 512], BF16)
                    if blk % 2 == 0:
                        nc.scalar.copy(fusedT[:, :nb], tp[:, :nb])
                    else:
                        nc.vector.tensor_copy(fusedT[:, :nb], tp[:, :nb])
                    mps = msgs_ps.tile([P, 512], F32, tag="mps")
                    for s in range(nblk):
                        nc.tensor.matmul(
                            mps[:, s * P:(s + 1) * P],
                            lhsT=fusedT[:, s * P:(s + 1) * P],
                            rhs=wfc, start=True, stop=True)
                    msgs = msgs_sb_pool.tile([P, 512], BF16)
                    nc.scalar.activation(msgs[:, :nb], mps[:, :nb],
                                         mybir.ActivationFunctionType.Tanh)
                    for s in range(nblk):
                        kk = k + s
                        S = s_pool.tile([P, P], BF16)
                        nc.vector.tensor_scalar(
                            out=S[:], in0=iota, scalar1=it[:, kk:kk + 1],
                            scalar2=None, op0=mybir.AluOpType.is_equal)
                        nc.tensor.matmul(
                            win[:], lhsT=S[:], rhs=msgs[:, s * P:(s + 1) * P],
                            start=(kk == 0), stop=(kk == TPW - 1))
                    k += nblk
                    blk += 1

                # ---- window flush ----
                afTo = fl_pool.tile([P, P], BF16, tag="afTo")
                nc.sync.dma_start(afTo[:], afT_own[:, w * P:(w + 1) * P])
                afo = fl_pool.tile([P, P], F32, tag="afo")
                nc.sync.dma_start(afo[:], af_own[w * P:(w + 1) * P, :])
                ah = dh_ps.tile([P, P], F32, tag="dh")
                nc.tensor.matmul(ah[:], lhsT=wcf, rhs=afTo[:],
                                 start=True, stop=False)
                nc.tensor.matmul(ah[:], lhsT=bcf, rhs=ones,
                                 start=False, stop=True)
                iipre = fl_pool.tile([P, P], BF16, tag="iipre")
                nc.vector.tensor_scalar(
                    out=iipre[:], in0=ah[:], scalar1=bdf, scalar2=None,
                    op0=mybir.AluOpType.mult)
                iips = msgs_ps.tile([P, P], F32, tag="mps")
                nc.tensor.matmul(iips[:], lhsT=iipre[:], rhs=wfc,
                                 start=True, stop=True)
                ii = fl_pool.tile([P, P], F32, tag="ii")
                nc.scalar.activation(ii[:], iips[:],
                                     mybir.ActivationFunctionType.Tanh)
                tmp = fl_pool.tile([P, P], F32, tag="tmp")
                nc.vector.tensor_tensor(tmp[:], win[:], ii[:],
                                        op=mybir.AluOpType.subtract)
                res = fl_pool.tile([P, P], F32, tag="res")
                nc.vector.tensor_tensor(res[:], tmp[:], afo[:],
                                        op=mybir.AluOpType.add)
                nc.sync.dma_start(out_d[w * P:(w + 1) * P, :], res[:])

    nc.compile()
    return nc


def _wrap16(ix):
    """idx n -> [n % 16, n // 16], replicated to 128 partitions."""
    a = np.ascontiguousarray(ix.reshape(-1, 16).T)          # [16, CAP//16]
    return np.tile(a, (8, 1))                               # [128, CAP//16]


def host_prep(atom_features, distance, atom_membership,
              distance_membership_i, distance_membership_j,
              W_cf, W_df, W_fc, b_cf, b_df):
    af = np.ascontiguousarray(atom_features, dtype=np.float32)
    dist = np.ascontiguousarray(distance, dtype=np.float32)
    i = np.ascontiguousarray(distance_membership_i, dtype=np.int64)
    j = np.ascontiguousarray(distance_membership_j, dtype=np.int64)

    afT_full = np.zeros((P, NTBL), NPBF)
    afT_full[:, :N_ATOMS] = af.T.astype(NPBF)
    wdfe = np.concatenate([np.asarray(W_df, np.float32),
                           np.asarray(b_df, np.float32)[None, :]], axis=0)
    cp16 = np.zeros((P, C16W), np.float32)
    cp16[:, 0:P] = np.asarray(W_cf, np.float32)
    cp16[:101, P:2 * P] = wdfe
    cp16[:, 2 * P:3 * P] = np.asarray(W_fc, np.float32)
    cp16[:, 3 * P:4 * P] = np.arange(P, dtype=np.float32)[None, :]
    cp16[:, 4 * P:5 * P] = np.eye(P, dtype=np.float32)
    cp32 = np.zeros((P, C32W), np.float32)
    cp32[:, 0] = np.asarray(b_df, np.float32)
    cp32[0, P:2 * P] = np.asarray(b_cf, np.float32)
    cp32[0, 2 * P:3 * P] = 1.0
    cp32[:, 3 * P:3 * P + 512] = np.tile(np.asarray(b_cf, np.float32), 4)[None, :]
    shared = {
        "afT": afT_full,
        "cp16": cp16.astype(NPBF),
        "cp32": cp32,
    }

    in_maps = []
    for c in range(NCORES):
        distT_c = np.zeros((101, NWIN * CAP), NPBF)
        distT_c[100, :] = 1.0
        j_c = np.zeros((NWIN, P, TPW), np.int32)
        ip_c = np.full((NWIN, P, TPW), -1.0, np.float32)
        # relayout to [P, NWIN*TPW] at the end
        for w in range(NWIN):
            B = c * APC + w * P
            E = min(B + P, (c + 1) * APC)
            pb = int(np.searchsorted(i, B))
            pe = int(np.searchsorted(i, E))
            n = pe - pb
            if n > CAP:
                raise AssertionError(f"window overflow: {n} > {CAP}")
            col0 = w * CAP
            distT_c[:100, col0:col0 + n] = dist[pb:pe].T.astype(NPBF)
            jw = np.zeros(CAP, np.int64)
            jw[:n] = j[pb:pe]
            jrow = np.where(jw < SPLIT, jw + 1, jw + 2).astype(np.int32)
            j_c[w] = jrow.reshape(TPW, P).T
            ipw = np.full(CAP, -1.0, np.float32)
            ipw[:n] = (i[pb:pe] - B).astype(np.float32)
            ip_c[w] = ipw.reshape(TPW, P).T
        af_own = np.zeros((APC_PAD, P), np.float32)
        af_own[:APC] = af[c * APC:(c + 1) * APC]
        m = {
            "distT": distT_c,
            "jidx": np.ascontiguousarray(
                j_c.transpose(1, 0, 2).reshape(P, NWIN * TPW)),
            "iprime": np.ascontiguousarray(
                ip_c.transpose(1, 0, 2).reshape(P, NWIN * TPW)),
            "af_own": af_own,
            "afT_own": np.ascontiguousarray(af_own.T).astype(NPBF),
        }
        m.update(shared)
        in_maps.append(m)
    return in_maps


_NC_CACHE = {}


def get_nc():
    if "nc" not in _NC_CACHE:
        _NC_CACHE["nc"] = build_nc()
    return _NC_CACHE["nc"]


def kernel(**inputs):
    in_maps = host_prep(**inputs)
    nc = get_nc()
    res = run_bass_kernel_spmd(nc, in_maps, core_ids=list(range(NCORES)))
    out = np.empty((N_ATOMS, N_EMB), np.float32)
    for c in range(NCORES):
        out[c * APC:(c + 1) * APC] = res.results[c]["out"][:APC]
    return out



# revision 3
# speedup vs baseline: 3.2232x; 3.2232x over previous
"""DTNNStep graph-message-passing kernel for 8x Trainium2 NeuronCores (v3).

Strategy: distance_membership_i is sorted, so pairs are sharded by
destination-atom range (6250 atoms per core). Each core processes 50
variable-width "windows" (<=128 atoms each, chosen on host so every
window holds <= 2048 pairs), so each window is exactly TPW=16 pair
tiles of 128 and the instruction stream is identical across cores.

The per-pair gather afh[j] is restructured: the HOST pre-gathers raw
atom_features[j] per pair (pure data layout) into a sequential bf16
stream gaT [128 emb, pairs]; the device computes
afh^T = W_cf^T @ gaT + b_cf per 512-pair block with a constant
stationary operand. This removes the device-side indirect-DMA gather,
the afh table build phase, and the per-tile PE transpose of the
baseline. dist^T is padded to 112 partitions (divisible by 16) so its
DMA stream sprays across all 16 DMA engines.

Pipeline per 512-pair block (all bf16 matmuls, f32 PSUM):
  dh^T  = wdfe^T @ distT-block          (PE, stationary wdfe const)
  afh^T = W_cf^T @ gaT-block            (PE, stationary W_cf const)
  biased = afh^T + b_cf                 (DVE, per-partition scalar)
  fusedT = biased * dh^T                (DVE)
  S[p,f] = (iota[f] == i'[p]) per tile  (DVE, one-hot dest-atom)
  mp     = fusedT-tile^T @ W_fc         (PE, per tile)
  msgs   = tanh(mp)                     (Scalar)
  win   += S^T @ msgs                   (PE PSUM accumulate, per tile)
Window flush computes the self-interaction term from the own-atom
columns appended to the gaT stream and writes out = win - ii + af.
No collectives: each core owns a disjoint output slice.
"""

import sys

for _p in ("/opt/trn_rl_repo",):
    if _p not in sys.path:
        sys.path.insert(0, _p)

import numpy as np
import ml_dtypes
import concourse.bass as bass
import concourse.bacc as bacc
import concourse.tile as tile
from concourse import mybir
from concourse.bass_utils import run_bass_kernel_spmd

F32 = mybir.dt.float32
BF16 = mybir.dt.bfloat16
NPBF = ml_dtypes.bfloat16

P = 128
N_ATOMS = 50000
N_PAIRS = 800000
N_EMB = 128
NCORES = 8
APC = N_ATOMS // NCORES            # atoms per core: 6250
TPW = 16                           # pair tiles per window
CAP = TPW * P                      # pair capacity per window: 2048
NWIN = 50                          # windows per core
NBLK = TPW // 4                    # 4-tile (512-pair) blocks per window
DPAD = 112                         # dist rows: 100 + bias row + pad (16|112)
GW = CAP + P                       # gaT cols per window (pairs + own atoms)
C16W = 4 * P                       # bf16 const pack width


def build_nc():
    nc = bacc.Bacc()

    distT = nc.declare_dram_parameter("distT", [DPAD, NWIN * CAP], BF16,
                                      isOutput=False)
    gaT_d = nc.declare_dram_parameter("gaT", [P, NWIN * GW], BF16,
                                      isOutput=False)
    iprm = nc.declare_dram_parameter("iprm", [P, NWIN * TPW], F32,
                                     isOutput=False)
    afo_d = nc.declare_dram_parameter("afo", [NWIN * P, P], F32,
                                      isOutput=False)
    cp16_d = nc.declare_dram_parameter("cp16", [P, C16W], BF16, isOutput=False)
    cp32_d = nc.declare_dram_parameter("cp32", [P, 2], F32, isOutput=False)
    out_d = nc.declare_dram_parameter("out", [NWIN * P, P], F32, isOutput=True)

    AT = mybir.AluOpType
    Tanh = mybir.ActivationFunctionType.Tanh

    with tile.TileContext(nc) as tc:
        with (
            tc.tile_pool(name="consts", bufs=1) as cpool,
            tc.tile_pool(name="dist", bufs=3) as dist_pool,
            tc.tile_pool(name="ga", bufs=3) as ga_pool,
            tc.tile_pool(name="afo", bufs=2) as afo_pool,
            tc.tile_pool(name="biased", bufs=3) as b_pool,
            tc.tile_pool(name="fusedT", bufs=3) as f_pool,
            tc.tile_pool(name="sgen", bufs=3) as s_pool,
            tc.tile_pool(name="msgs", bufs=3) as m_pool,
            tc.tile_pool(name="flush", bufs=2) as fl_pool,
            tc.tile_pool(name="ps_dh", bufs=2, space="PSUM") as dh_ps,
            tc.tile_pool(name="ps_ah", bufs=2, space="PSUM") as ah_ps,
            tc.tile_pool(name="ps_m", bufs=2, space="PSUM") as m_ps,
            tc.tile_pool(name="ps_win", bufs=2, space="PSUM") as win_ps,
        ):
            cpk = cpool.tile([P, C16W], BF16)
            nc.sync.dma_start(cpk[:], cp16_d[:])
            wcf = cpk[:, 0:P]
            wfc = cpk[:, P:2 * P]
            wdfe = cpk[:DPAD, 2 * P:3 * P]
            iota = cpk[:, 3 * P:4 * P]
            cpk32 = cpool.tile([P, 2], F32)
            nc.sync.dma_start(cpk32[:], cp32_d[:])
            bcf = cpk32[:, 0:1]
            bdf = cpk32[:, 1:2]
            iall = cpool.tile([P, NWIN * TPW], F32)
            nc.gpsimd.dma_start(iall[:], iprm[:])

            for w in range(NWIN):
                dt = dist_pool.tile([DPAD, CAP], BF16)
                nc.sync.dma_start(dt[:], distT[:, w * CAP:(w + 1) * CAP])
                ga = ga_pool.tile([P, GW], BF16)
                nc.scalar.dma_start(ga[:], gaT_d[:, w * GW:(w + 1) * GW])
                afo = afo_pool.tile([P, P], F32)
                nc.gpsimd.dma_start(afo[:], afo_d[w * P:(w + 1) * P, :])

                win = win_ps.tile([P, P], F32)

                # prologue: first block's matmuls + flush afh so PE can
                # run ahead of the DVE/Scalar stages
                ah0 = ah_ps.tile([P, 512], F32, tag="ah")
                nc.tensor.matmul(ah0[:], lhsT=wcf, rhs=ga[:, 0:512],
                                 start=True, stop=True)
                dh0 = dh_ps.tile([P, 512], F32, tag="dh")
                nc.tensor.matmul(dh0[:], lhsT=wdfe, rhs=dt[:, 0:512],
                                 start=True, stop=True)
                aho = m_ps.tile([P, P], F32, tag="mp")
                nc.tensor.matmul(aho[:], lhsT=wcf, rhs=ga[:, CAP:CAP + P],
                                 start=True, stop=True)
                ipre = fl_pool.tile([P, P], BF16, tag="ipre")
                nc.vector.tensor_scalar(out=ipre[:], in0=aho[:], scalar1=bcf,
                                        scalar2=bdf, op0=AT.add, op1=AT.mult)

                ah, dh = ah0, dh0
                for b in range(NBLK):
                    # issue next block's heavy matmuls first (pipelining)
                    if b + 1 < NBLK:
                        c1 = (b + 1) * 512
                        ahn = ah_ps.tile([P, 512], F32, tag="ah")
                        nc.tensor.matmul(ahn[:], lhsT=wcf,
                                         rhs=ga[:, c1:c1 + 512],
                                         start=True, stop=True)
                        dhn = dh_ps.tile([P, 512], F32, tag="dh")
                        nc.tensor.matmul(dhn[:], lhsT=wdfe,
                                         rhs=dt[:, c1:c1 + 512],
                                         start=True, stop=True)
                    biased = b_pool.tile([P, 512], BF16)
                    nc.vector.tensor_scalar(out=biased[:], in0=ah[:],
                                            scalar1=bcf, scalar2=None,
                                            op0=AT.add)
                    fusedT = f_pool.tile([P, 512], BF16)
                    nc.vector.tensor_tensor(fusedT[:], biased[:], dh[:],
                                            op=AT.mult)
                    S4 = s_pool.tile([P, 512], BF16)
                    for s in range(4):
                        col = w * TPW + b * 4 + s
                        nc.vector.tensor_scalar(
                            out=S4[:, s * P:(s + 1) * P], in0=iota,
                            scalar1=iall[:, col:col + 1], scalar2=None,
                            op0=AT.is_equal)
                    mp = m_ps.tile([P, 512], F32, tag="mp")
                    for s in range(4):
                        nc.tensor.matmul(mp[:, s * P:(s + 1) * P],
                                         lhsT=fusedT[:, s * P:(s + 1) * P],
                                         rhs=wfc, start=True, stop=True)
                    msgs = m_pool.tile([P, 512], BF16)
                    nc.scalar.activation(msgs[:], mp[:], Tanh)
                    for s in range(4):
                        kk = b * 4 + s
                        nc.tensor.matmul(win[:],
                                         lhsT=S4[:, s * P:(s + 1) * P],
                                         rhs=msgs[:, s * P:(s + 1) * P],
                                         start=(kk == 0), stop=(kk == TPW - 1))
                    if b + 1 < NBLK:
                        ah, dh = ahn, dhn

                # ---- window flush ----
                iips = m_ps.tile([P, P], F32, tag="mp")
                nc.tensor.matmul(iips[:], lhsT=ipre[:], rhs=wfc,
                                 start=True, stop=True)
                ii = fl_pool.tile([P, P], F32, tag="ii")
                nc.scalar.activation(ii[:], iips[:], Tanh)
                tmp = fl_pool.tile([P, P], F32, tag="tmp")
                nc.vector.tensor_tensor(tmp[:], win[:], ii[:],
                                        op=AT.subtract)
                res = fl_pool.tile([P, P], F32, tag="res")
                nc.vector.tensor_tensor(res[:], tmp[:], afo[:], op=AT.add)
                nc.sync.dma_start(out_d[w * P:(w + 1) * P, :], res[:])

    nc.compile()
    return nc


def host_prep(atom_features, distance, atom_membership,
              distance_membership_i, distance_membership_j,
              W_cf, W_df, W_fc, b_cf, b_df):
    """Pack per-core inputs. Returns (in_maps, outmaps) where outmaps[c]
    maps each core-local atom row to its padded out-tensor row."""
    af = np.asarray(atom_features, np.float32)
    i = np.asarray(distance_membership_i, np.int64)
    j = np.asarray(distance_membership_j, np.int64)
    dist_bf = np.asarray(distance, np.float32).astype(NPBF)
    af_bf = af.astype(NPBF)
    af_ext = np.concatenate([af_bf, np.zeros((1, P), NPBF)], axis=0)
    counts = np.bincount(i, minlength=N_ATOMS)

    wdfe = np.zeros((DPAD, P), np.float32)
    wdfe[:100] = np.asarray(W_df, np.float32)
    wdfe[100] = np.asarray(b_df, np.float32)
    cp16 = np.zeros((P, C16W), np.float32)
    cp16[:, 0:P] = np.asarray(W_cf, np.float32)
    cp16[:, P:2 * P] = np.asarray(W_fc, np.float32)
    cp16[:DPAD, 2 * P:3 * P] = wdfe
    cp16[:, 3 * P:4 * P] = np.arange(P, dtype=np.float32)[None, :]
    cp32 = np.zeros((P, 2), np.float32)
    cp32[:, 0] = np.asarray(b_cf, np.float32)
    cp32[:, 1] = np.asarray(b_df, np.float32)
    shared = {"cp16": cp16.astype(NPBF), "cp32": cp32}

    in_maps = []
    outmaps = []
    for c in range(NCORES):
        a_lo, a_hi = c * APC, (c + 1) * APC
        cnt = counts[a_lo:a_hi]
        # greedy max-fill: window takes atoms while <=128 atoms & <=CAP pairs
        bounds = [0]
        pos = 0
        while pos < APC:
            take, s = 0, 0
            while take < P and pos + take < APC and \
                    s + cnt[pos + take] <= CAP:
                s += cnt[pos + take]
                take += 1
            assert take > 0, "single atom exceeds window capacity"
            pos += take
            bounds.append(pos)
        assert len(bounds) - 1 <= NWIN, f"needs {len(bounds)-1} windows"
        while len(bounds) < NWIN + 1:
            bounds.append(APC)
        bounds = np.asarray(bounds, np.int64) + a_lo
        pb = np.searchsorted(i, bounds)
        npair = pb[1:] - pb[:-1]
        natom = bounds[1:] - bounds[:-1]
        assert npair.max() <= CAP

        colmap = np.full((NWIN, CAP), -1, np.int64)
        jmap = np.full((NWIN, GW), N_ATOMS, np.int64)
        ipr = np.full((NWIN, CAP), -1.0, np.float32)
        for w in range(NWIN):
            n = int(npair[w])
            colmap[w, :n] = np.arange(pb[w], pb[w + 1])
            jmap[w, :n] = j[pb[w]:pb[w + 1]]
            jmap[w, CAP:CAP + natom[w]] = np.arange(bounds[w], bounds[w + 1])
            ipr[w, :n] = (i[pb[w]:pb[w + 1]] - bounds[w]).astype(np.float32)

        flat = colmap.reshape(-1)
        m = flat >= 0
        dT = np.zeros((NWIN * CAP, DPAD), NPBF)
        dT[m, :100] = dist_bf[flat[m]]
        dT[m, 100] = 1.0
        distT_c = np.ascontiguousarray(dT.T)

        gaT_c = np.ascontiguousarray(af_ext[jmap.reshape(-1)].T)

        iprm_c = np.ascontiguousarray(
            ipr.reshape(NWIN, TPW, P).transpose(2, 0, 1).reshape(P, NWIN * TPW))

        rowmap = np.full((NWIN, P), -1, np.int64)
        outmap = np.empty(APC, np.int64)
        for w in range(NWIN):
            na = int(natom[w])
            rowmap[w, :na] = np.arange(bounds[w], bounds[w + 1])
            outmap[bounds[w] - a_lo:bounds[w + 1] - a_lo] = \
                w * P + np.arange(na)
        rflat = rowmap.reshape(-1)
        rm = rflat >= 0
        afo_c = np.zeros((NWIN * P, P), np.float32)
        afo_c[rm] = af[rflat[rm]]

        mdict = {
            "distT": distT_c,
            "gaT": gaT_c,
            "iprm": iprm_c,
            "afo": afo_c,
        }
        mdict.update(shared)
        in_maps.append(mdict)
        outmaps.append(outmap)
    return in_maps, outmaps


def unshard(results, outmaps):
    out = np.empty((N_ATOMS, N_EMB), np.float32)
    for c in range(NCORES):
        out[c * APC:(c + 1) * APC] = results[c]["out"][outmaps[c]]
    return out


_NC_CACHE = {}


def get_nc():
    if "nc" not in _NC_CACHE:
        _NC_CACHE["nc"] = build_nc()
    return _NC_CACHE["nc"]


def kernel(**inputs):
    in_maps, outmaps = host_prep(**inputs)
    nc = get_nc()
    res = run_bass_kernel_spmd(nc, in_maps, core_ids=list(range(NCORES)))
    return unshard(res.results, outmaps)


# revision 10
# speedup vs baseline: 3.2723x; 1.0152x over previous
"""DTNNStep graph-message-passing kernel for 8x Trainium2 NeuronCores (v3).

Strategy: distance_membership_i is sorted, so pairs are sharded by
destination-atom range (6250 atoms per core). Each core processes 50
variable-width "windows" (<=128 atoms each, chosen on host so every
window holds <= 2048 pairs), so each window is exactly TPW=16 pair
tiles of 128 and the instruction stream is identical across cores.

The per-pair gather afh[j] is restructured: the HOST pre-gathers raw
atom_features[j] per pair (pure data layout) into a sequential bf16
stream gaT [128 emb, pairs]; the device computes
afh^T = W_cf^T @ gaT + b_cf per 512-pair block with a constant
stationary operand. This removes the device-side indirect-DMA gather,
the afh table build phase, and the per-tile PE transpose of the
baseline. dist^T is padded to 112 partitions (divisible by 16) so its
DMA stream sprays across all 16 DMA engines.

Pipeline per 512-pair block (all bf16 matmuls, f32 PSUM):
  dh^T  = wdfe^T @ distT-block          (PE, stationary wdfe const)
  afh^T = W_cf^T @ gaT-block            (PE, stationary W_cf const)
  biased = afh^T + b_cf                 (DVE, per-partition scalar)
  fusedT = biased * dh^T                (DVE)
  S[p,f] = (iota[f] == i'[p]) per tile  (DVE, one-hot dest-atom)
  mp     = fusedT-tile^T @ W_fc         (PE, per tile)
  msgs   = tanh(mp)                     (Scalar)
  win   += S^T @ msgs                   (PE PSUM accumulate, per tile)
Window flush computes the self-interaction term from the own-atom
columns appended to the gaT stream and writes out = win - ii + af.
No collectives: each core owns a disjoint output slice.
"""

import sys

for _p in ("/opt/trn_rl_repo",):
    if _p not in sys.path:
        sys.path.insert(0, _p)

import numpy as np
import ml_dtypes
import concourse.bass as bass
import concourse.bacc as bacc
import concourse.tile as tile
from concourse import mybir
from concourse.bass_utils import run_bass_kernel_spmd

F32 = mybir.dt.float32
BF16 = mybir.dt.bfloat16
NPBF = ml_dtypes.bfloat16

P = 128
N_ATOMS = 50000
N_PAIRS = 800000
N_EMB = 128
NCORES = 8
APC = N_ATOMS // NCORES            # atoms per core: 6250
TPW = 16                           # pair tiles per window
CAP = TPW * P                      # pair capacity per window: 2048
NWIN = 50                          # windows per core
NBLK = TPW // 4                    # 4-tile (512-pair) blocks per window
DPAD = 112                         # dist rows: 100 + bias row + pad (16|112)
GW = CAP + P                       # gaT cols per window (pairs + own atoms)
C16W = 5 * P                       # bf16 const pack width


def build_nc():
    nc = bacc.Bacc()

    distT = nc.declare_dram_parameter("distT", [DPAD, NWIN * CAP], BF16,
                                      isOutput=False)
    gaT_d = nc.declare_dram_parameter("gaT", [P, NWIN * GW], BF16,
                                      isOutput=False)
    iprm = nc.declare_dram_parameter("iprm", [P, NWIN * TPW], BF16,
                                     isOutput=False)
    afo_d = nc.declare_dram_parameter("afo", [NWIN * P, P], BF16,
                                      isOutput=False)
    cp16_d = nc.declare_dram_parameter("cp16", [P, C16W], BF16, isOutput=False)
    cp32_d = nc.declare_dram_parameter("cp32", [P, 2], F32, isOutput=False)
    out_d = nc.declare_dram_parameter("out", [NWIN * P, P], F32, isOutput=True)

    AT = mybir.AluOpType
    Tanh = mybir.ActivationFunctionType.Tanh
    Ident = mybir.ActivationFunctionType.Identity

    with tile.TileContext(nc) as tc:
        with (
            tc.tile_pool(name="consts", bufs=1) as cpool,
            tc.tile_pool(name="dist", bufs=3) as dist_pool,
            tc.tile_pool(name="ga", bufs=3) as ga_pool,
            tc.tile_pool(name="afo", bufs=2) as afo_pool,
            tc.tile_pool(name="biased", bufs=3) as b_pool,
            tc.tile_pool(name="fusedT", bufs=3) as f_pool,
            tc.tile_pool(name="sgen", bufs=3) as s_pool,
            tc.tile_pool(name="msgs", bufs=3) as m_pool,
            tc.tile_pool(name="flush", bufs=2) as fl_pool,
            tc.tile_pool(name="ps_dh", bufs=2, space="PSUM") as dh_ps,
            tc.tile_pool(name="ps_ah", bufs=2, space="PSUM") as ah_ps,
            tc.tile_pool(name="ps_m", bufs=2, space="PSUM") as m_ps,
            tc.tile_pool(name="ps_win", bufs=2, space="PSUM") as win_ps,
        ):
            cpk = cpool.tile([P, C16W], BF16)
            nc.sync.dma_start(cpk[:], cp16_d[:])
            wcf = cpk[:, 0:P]
            wfc = cpk[:, P:2 * P]
            wdfe = cpk[:DPAD, 2 * P:3 * P]
            iota = cpk[:, 3 * P:4 * P]
            ident = cpk[:, 4 * P:5 * P]
            cpk32 = cpool.tile([P, 2], F32)
            nc.sync.dma_start(cpk32[:], cp32_d[:])
            bcf = cpk32[:, 0:1]
            bdf = cpk32[:, 1:2]
            iall = cpool.tile([P, NWIN * TPW], BF16)
            nc.gpsimd.dma_start(iall[:], iprm[:])
            # iota broadcast over the 4-tile dim: [128, 4(x0), 128]
            iotaB = bass.AP(iota.tensor, iota.offset,
                            [list(iota.ap[0]), [0, 4], list(iota.ap[1])])

            for w in range(NWIN):
                dt = dist_pool.tile([DPAD, CAP], BF16)
                nc.sync.dma_start(dt[:], distT[:, w * CAP:(w + 1) * CAP])
                ga = ga_pool.tile([P, GW], BF16)
                nc.scalar.dma_start(ga[:], gaT_d[:, w * GW:(w + 1) * GW])
                afo = afo_pool.tile([P, P], BF16)
                nc.gpsimd.dma_start(afo[:], afo_d[w * P:(w + 1) * P, :])

                win = win_ps.tile([P, P], F32)
                # residual init: win = I^T @ afo
                nc.tensor.matmul(win[:], lhsT=ident, rhs=afo[:],
                                 start=True, stop=False)

                # prologue: first block's matmuls + flush afh so PE can
                # run ahead of the DVE/Scalar stages
                ah0 = ah_ps.tile([P, 512], F32, tag="ah")
                nc.tensor.matmul(ah0[:], lhsT=wcf, rhs=ga[:, 0:512],
                                 start=True, stop=True)
                dh0 = dh_ps.tile([P, 512], F32, tag="dh")
                nc.tensor.matmul(dh0[:], lhsT=wdfe, rhs=dt[:, 0:512],
                                 start=True, stop=True)
                aho = m_ps.tile([P, P], F32, tag="mp")
                nc.tensor.matmul(aho[:], lhsT=wcf, rhs=ga[:, CAP:CAP + P],
                                 start=True, stop=True)
                ipre = fl_pool.tile([P, P], BF16, tag="ipre")
                nc.vector.tensor_scalar(out=ipre[:], in0=aho[:], scalar1=bcf,
                                        scalar2=bdf, op0=AT.add, op1=AT.mult)

                ah, dh = ah0, dh0
                for b in range(NBLK):
                    # issue next block's heavy matmuls first (pipelining)
                    if b + 1 < NBLK:
                        c1 = (b + 1) * 512
                        ahn = ah_ps.tile([P, 512], F32, tag="ah")
                        nc.tensor.matmul(ahn[:], lhsT=wcf,
                                         rhs=ga[:, c1:c1 + 512],
                                         start=True, stop=True)
                        dhn = dh_ps.tile([P, 512], F32, tag="dh")
                        nc.tensor.matmul(dhn[:], lhsT=wdfe,
                                         rhs=dt[:, c1:c1 + 512],
                                         start=True, stop=True)
                    biased = b_pool.tile([P, 512], BF16)
                    nc.scalar.activation(biased[:], ah[:], Ident, bias=bcf)
                    fusedT = f_pool.tile([P, 512], BF16)
                    nc.vector.tensor_tensor(fusedT[:], biased[:], dh[:],
                                            op=AT.mult)
                    S4 = s_pool.tile([P, 512], BF16)
                    col = w * TPW + b * 4
                    it4 = iall[:, col:col + 4]
                    it4B = bass.AP(it4.tensor, it4.offset,
                                   [list(it4.ap[0]), list(it4.ap[1]),
                                    [0, P]])
                    S4v = S4[:].rearrange("p (s f) -> p s f", f=P)
                    nc.vector.tensor_tensor(S4v, iotaB, it4B,
                                            op=AT.is_equal)
                    mp = m_ps.tile([P, 512], F32, tag="mp")
                    for s in range(4):
                        nc.tensor.matmul(mp[:, s * P:(s + 1) * P],
                                         lhsT=fusedT[:, s * P:(s + 1) * P],
                                         rhs=wfc, start=True, stop=True)
                    msgs = m_pool.tile([P, 512], BF16)
                    nc.scalar.activation(msgs[:], mp[:], Tanh)
                    for s in range(4):
                        kk = b * 4 + s
                        nc.tensor.matmul(win[:],
                                         lhsT=S4[:, s * P:(s + 1) * P],
                                         rhs=msgs[:, s * P:(s + 1) * P],
                                         start=False, stop=(kk == TPW - 1))
                    if b + 1 < NBLK:
                        ah, dh = ahn, dhn

                # ---- window flush ----
                iips = m_ps.tile([P, P], F32, tag="mp")
                nc.tensor.matmul(iips[:], lhsT=ipre[:], rhs=wfc,
                                 start=True, stop=True)
                ii = fl_pool.tile([P, P], F32, tag="ii")
                nc.scalar.activation(ii[:], iips[:], Tanh)
                res = fl_pool.tile([P, P], F32, tag="res")
                nc.vector.tensor_tensor(res[:], win[:], ii[:],
                                        op=AT.subtract)
                nc.sync.dma_start(out_d[w * P:(w + 1) * P, :], res[:])

    nc.compile()
    return nc


def host_prep(atom_features, distance, atom_membership,
              distance_membership_i, distance_membership_j,
              W_cf, W_df, W_fc, b_cf, b_df):
    """Pack per-core inputs. Returns (in_maps, outmaps) where outmaps[c]
    maps each core-local atom row to its padded out-tensor row."""
    af = np.asarray(atom_features, np.float32)
    i = np.asarray(distance_membership_i, np.int64)
    j = np.asarray(distance_membership_j, np.int64)
    dist_bf = np.asarray(distance, np.float32).astype(NPBF)
    af_bf = af.astype(NPBF)
    af_ext = np.concatenate([af_bf, np.zeros((1, P), NPBF)], axis=0)
    counts = np.bincount(i, minlength=N_ATOMS)

    wdfe = np.zeros((DPAD, P), np.float32)
    wdfe[:100] = np.asarray(W_df, np.float32)
    wdfe[100] = np.asarray(b_df, np.float32)
    cp16 = np.zeros((P, C16W), np.float32)
    cp16[:, 0:P] = np.asarray(W_cf, np.float32)
    cp16[:, P:2 * P] = np.asarray(W_fc, np.float32)
    cp16[:DPAD, 2 * P:3 * P] = wdfe
    cp16[:, 3 * P:4 * P] = np.arange(P, dtype=np.float32)[None, :]
    cp16[:, 4 * P:5 * P] = np.eye(P, dtype=np.float32)
    cp32 = np.zeros((P, 2), np.float32)
    cp32[:, 0] = np.asarray(b_cf, np.float32)
    cp32[:, 1] = np.asarray(b_df, np.float32)
    shared = {"cp16": cp16.astype(NPBF), "cp32": cp32}

    in_maps = []
    outmaps = []
    for c in range(NCORES):
        a_lo, a_hi = c * APC, (c + 1) * APC
        cnt = counts[a_lo:a_hi]
        # greedy max-fill: window takes atoms while <=128 atoms & <=CAP pairs
        bounds = [0]
        pos = 0
        while pos < APC:
            take, s = 0, 0
            while take < P and pos + take < APC and \
                    s + cnt[pos + take] <= CAP:
                s += cnt[pos + take]
                take += 1
            assert take > 0, "single atom exceeds window capacity"
            pos += take
            bounds.append(pos)
        assert len(bounds) - 1 <= NWIN, f"needs {len(bounds)-1} windows"
        while len(bounds) < NWIN + 1:
            bounds.append(APC)
        bounds = np.asarray(bounds, np.int64) + a_lo
        pb = np.searchsorted(i, bounds)
        npair = pb[1:] - pb[:-1]
        natom = bounds[1:] - bounds[:-1]
        assert npair.max() <= CAP

        colmap = np.full((NWIN, CAP), -1, np.int64)
        jmap = np.full((NWIN, GW), N_ATOMS, np.int64)
        ipr = np.full((NWIN, CAP), -1.0, np.float32)
        for w in range(NWIN):
            n = int(npair[w])
            colmap[w, :n] = np.arange(pb[w], pb[w + 1])
            jmap[w, :n] = j[pb[w]:pb[w + 1]]
            jmap[w, CAP:CAP + natom[w]] = np.arange(bounds[w], bounds[w + 1])
            ipr[w, :n] = (i[pb[w]:pb[w + 1]] - bounds[w]).astype(np.float32)

        flat = colmap.reshape(-1)
        m = flat >= 0
        dT = np.zeros((NWIN * CAP, DPAD), NPBF)
        dT[m, :100] = dist_bf[flat[m]]
        dT[m, 100] = 1.0
        distT_c = np.ascontiguousarray(dT.T)

        gaT_c = np.ascontiguousarray(af_ext[jmap.reshape(-1)].T)

        iprm_c = np.ascontiguousarray(
            ipr.reshape(NWIN, TPW, P).transpose(2, 0, 1)
            .reshape(P, NWIN * TPW)).astype(NPBF)

        rowmap = np.full((NWIN, P), -1, np.int64)
        outmap = np.empty(APC, np.int64)
        for w in range(NWIN):
            na = int(natom[w])
            rowmap[w, :na] = np.arange(bounds[w], bounds[w + 1])
            outmap[bounds[w] - a_lo:bounds[w + 1] - a_lo] = \
                w * P + np.arange(na)
        rflat = rowmap.reshape(-1)
        rm = rflat >= 0
        afo_c = np.zeros((NWIN * P, P), NPBF)
        afo_c[rm] = af_bf[rflat[rm]]

        mdict = {
            "distT": distT_c,
            "gaT": gaT_c,
            "iprm": iprm_c,
            "afo": afo_c,
        }
        mdict.update(shared)
        in_maps.append(mdict)
        outmaps.append(outmap)
    return in_maps, outmaps


def unshard(results, outmaps):
    out = np.empty((N_ATOMS, N_EMB), np.float32)
    for c in range(NCORES):
        out[c * APC:(c + 1) * APC] = results[c]["out"][outmaps[c]]
    return out


_NC_CACHE = {}


def get_nc():
    if "nc" not in _NC_CACHE:
        _NC_CACHE["nc"] = build_nc()
    return _NC_CACHE["nc"]


def kernel(**inputs):
    in_maps, outmaps = host_prep(**inputs)
    nc = get_nc()
    res = run_bass_kernel_spmd(nc, in_maps, core_ids=list(range(NCORES)))
    return unshard(res.results, outmaps)


# revision 26
# speedup vs baseline: 4.3967x; 1.3436x over previous
"""DTNNStep graph-message-passing kernel for 8x Trainium2 NeuronCores (v3).

Strategy: distance_membership_i is sorted, so pairs are sharded by
destination-atom range (6250 atoms per core). Each core processes 50
variable-width "windows" (<=128 atoms each, chosen on host so every
window holds <= 2048 pairs), so each window is exactly TPW=16 pair
tiles of 128 and the instruction stream is identical across cores.

The per-pair gather afh[j] is restructured: the HOST pre-gathers raw
atom_features[j] per pair (pure data layout) into a sequential bf16
stream gaT [128 emb, pairs]; the device computes
afh^T = W_cf^T @ gaT + b_cf per 512-pair block with a constant
stationary operand. This removes the device-side indirect-DMA gather,
the afh table build phase, and the per-tile PE transpose of the
baseline. dist^T is padded to 112 partitions (divisible by 16) so its
DMA stream sprays across all 16 DMA engines.

Pipeline per 512-pair block (all bf16 matmuls, f32 PSUM):
  dh^T  = wdfe^T @ distT-block          (PE, stationary wdfe const)
  afh^T = W_cf^T @ gaT-block            (PE, stationary W_cf const)
  biased = afh^T + b_cf                 (DVE, per-partition scalar)
  fusedT = biased * dh^T                (DVE)
  S[p,f] = (iota[f] == i'[p]) per tile  (DVE, one-hot dest-atom)
  mp     = fusedT-tile^T @ W_fc         (PE, per tile)
  msgs   = tanh(mp)                     (Scalar)
  win   += S^T @ msgs                   (PE PSUM accumulate, per tile)
Window flush computes the self-interaction term from the own-atom
columns appended to the gaT stream and writes out = win - ii + af.
No collectives: each core owns a disjoint output slice.
"""

import sys

for _p in ("/opt/trn_rl_repo",):
    if _p not in sys.path:
        sys.path.insert(0, _p)

import numpy as np
import ml_dtypes
import concourse.bass as bass
import concourse.bacc as bacc
import concourse.tile as tile
from concourse import mybir
from concourse.bass_utils import run_bass_kernel_spmd

F32 = mybir.dt.float32
BF16 = mybir.dt.bfloat16
NPBF = ml_dtypes.bfloat16

P = 128
N_ATOMS = 50000
N_PAIRS = 800000
N_EMB = 128
NCORES = 8
APC = N_ATOMS // NCORES            # atoms per core: 6250
TPW = 16                           # pair tiles per window
CAP = TPW * P                      # pair capacity per window: 2048
NWIN = 50                          # windows per core
NBLK = TPW // 4                    # 4-tile (512-pair) blocks per window
DPAD = 112                         # dist rows: 100 + bias row + pad (16|112)
GW = CAP + P                       # gaT cols per window (pairs + own atoms)
C16W = 5 * P + 640                 # bf16 const pack width


def build_nc():
    nc = bacc.Bacc()

    distT = nc.declare_dram_parameter("distT", [DPAD, NWIN * CAP], BF16,
                                      isOutput=False)
    gaT_d = nc.declare_dram_parameter("gaT", [P, NWIN * GW], BF16,
                                      isOutput=False)
    S_d = nc.declare_dram_parameter("Sst", [P, NWIN * CAP], BF16,
                                    isOutput=False)
    afo_d = nc.declare_dram_parameter("afo", [NWIN * P, P], BF16,
                                      isOutput=False)
    cp16_d = nc.declare_dram_parameter("cp16", [P, C16W], BF16, isOutput=False)
    cp32_d = nc.declare_dram_parameter("cp32", [P, 2], F32, isOutput=False)
    out_d = nc.declare_dram_parameter("out", [NWIN * P, P], F32, isOutput=True)

    AT = mybir.AluOpType
    Tanh = mybir.ActivationFunctionType.Tanh
    Ident = mybir.ActivationFunctionType.Identity

    with tile.TileContext(nc) as tc:
        with (
            tc.tile_pool(name="consts", bufs=1) as cpool,
            tc.tile_pool(name="dist", bufs=3) as dist_pool,
            tc.tile_pool(name="ga", bufs=3) as ga_pool,
            tc.tile_pool(name="afo", bufs=3) as afo_pool,
            tc.tile_pool(name="biased", bufs=3) as b_pool,
            tc.tile_pool(name="fusedT", bufs=3) as f_pool,
            tc.tile_pool(name="sgen", bufs=2) as s_pool,
            tc.tile_pool(name="msgs", bufs=3) as m_pool,
            tc.tile_pool(name="flush", bufs=2) as fl_pool,
            tc.tile_pool(name="ps_dh", bufs=2, space="PSUM") as dh_ps,
            tc.tile_pool(name="ps_ah", bufs=2, space="PSUM") as ah_ps,
            tc.tile_pool(name="ps_m", bufs=2, space="PSUM") as m_ps,
            tc.tile_pool(name="ps_win", bufs=2, space="PSUM") as win_ps,
        ):
            fl_ps = m_ps
            cpk = cpool.tile([P, C16W], BF16)
            nc.sync.dma_start(cpk[:], cp16_d[:])
            wcf = cpk[:, 0:P]
            wfc = cpk[:, P:2 * P]
            wdfe = cpk[:DPAD, 2 * P:3 * P]
            iota = cpk[:, 3 * P:4 * P]
            ident = cpk[:, 4 * P:5 * P]
            bcfr = cpk[0:1, 5 * P:5 * P + P]
            ones = cpk[0:1, 5 * P + P:5 * P + 640]
            cpk32 = cpool.tile([P, 2], F32)
            nc.sync.dma_start(cpk32[:], cp32_d[:])
            bcf = cpk32[:, 0:1]
            bdf = cpk32[:, 1:2]
            for w in range(NWIN):
                dt = dist_pool.tile([DPAD, CAP], BF16)
                nc.sync.dma_start(dt[:], distT[:, w * CAP:(w + 1) * CAP])
                ga = ga_pool.tile([P, GW], BF16)
                nc.scalar.dma_start(ga[:], gaT_d[:, w * GW:(w + 1) * GW])
                St = s_pool.tile([P, CAP], BF16)
                nc.gpsimd.dma_start(St[:], S_d[:, w * CAP:(w + 1) * CAP])
                afo = afo_pool.tile([P, P], BF16)
                nc.gpsimd.dma_start(afo[:], afo_d[w * P:(w + 1) * P, :])

                win = win_ps.tile([P, P], F32)
                # residual init: win = I^T @ afo
                nc.tensor.matmul(win[:], lhsT=ident, rhs=afo[:],
                                 start=True, stop=False)

                # prologue: first block's matmuls + flush afh so PE can
                # run ahead of the DVE/Scalar stages
                ah0 = ah_ps.tile([P, 512], F32, tag="ah")
                nc.tensor.matmul(ah0[:], lhsT=wcf, rhs=ga[:, 0:512],
                                 start=True, stop=True)
                dh0 = dh_ps.tile([P, 512], F32, tag="dh")
                nc.tensor.matmul(dh0[:], lhsT=wdfe, rhs=dt[:, 0:512],
                                 start=True, stop=True)
                aho = fl_ps.tile([P, P], F32, tag="mp")
                nc.tensor.matmul(aho[:], lhsT=wcf, rhs=ga[:, CAP:CAP + P],
                                 start=True, stop=True)
                ipre = fl_pool.tile([P, P], BF16, tag="ipre")
                nc.vector.tensor_scalar(out=ipre[:], in0=aho[:], scalar1=bcf,
                                        scalar2=bdf, op0=AT.add, op1=AT.mult)

                ah, dh = ah0, dh0
                for b in range(NBLK):
                    # issue next block's heavy matmuls first (pipelining)
                    if b + 1 < NBLK:
                        c1 = (b + 1) * 512
                        ahn = ah_ps.tile([P, 512], F32, tag="ah")
                        nc.tensor.matmul(ahn[:], lhsT=wcf,
                                         rhs=ga[:, c1:c1 + 512],
                                         start=True, stop=True)
                        dhn = dh_ps.tile([P, 512], F32, tag="dh")
                        nc.tensor.matmul(dhn[:], lhsT=wdfe,
                                         rhs=dt[:, c1:c1 + 512],
                                         start=True, stop=True)
                    biased = b_pool.tile([P, 512], BF16)
                    if b % 2 == 0:
                        nc.scalar.activation(biased[:], ah[:], Ident,
                                             bias=bcf)
                    else:
                        nc.vector.tensor_scalar(out=biased[:], in0=ah[:],
                                                scalar1=bcf, scalar2=None,
                                                op0=AT.add)
                    fusedT = f_pool.tile([P, 512], BF16)
                    nc.vector.tensor_tensor(fusedT[:], biased[:], dh[:],
                                            op=AT.mult)
                    mp = m_ps.tile([P, 512], F32, tag="mp")
                    for s in range(4):
                        nc.tensor.matmul(mp[:, s * P:(s + 1) * P],
                                         lhsT=fusedT[:, s * P:(s + 1) * P],
                                         rhs=wfc, start=True, stop=True)
                    msgs = m_pool.tile([P, 512], BF16)
                    nc.scalar.activation(msgs[:], mp[:], Tanh)
                    for s in range(4):
                        kk = b * 4 + s
                        nc.tensor.matmul(win[:],
                                         lhsT=St[:, kk * P:(kk + 1) * P],
                                         rhs=msgs[:, s * P:(s + 1) * P],
                                         start=False, stop=(kk == TPW - 1))
                    if b + 1 < NBLK:
                        ah, dh = ahn, dhn

                # ---- window flush ----
                iips = fl_ps.tile([P, P], F32, tag="mp")
                nc.tensor.matmul(iips[:], lhsT=ipre[:], rhs=wfc,
                                 start=True, stop=True)
                ii = fl_pool.tile([P, P], F32, tag="ii")
                nc.scalar.activation(ii[:], iips[:], Tanh)
                res = fl_pool.tile([P, P], F32, tag="res")
                nc.vector.tensor_tensor(res[:], win[:], ii[:],
                                        op=AT.subtract)
                nc.sync.dma_start(out_d[w * P:(w + 1) * P, :], res[:])

    nc.compile()
    return nc


def host_prep(atom_features, distance, atom_membership,
              distance_membership_i, distance_membership_j,
              W_cf, W_df, W_fc, b_cf, b_df):
    """Pack per-core inputs. Returns (in_maps, outmaps) where outmaps[c]
    maps each core-local atom row to its padded out-tensor row."""
    af = np.asarray(atom_features, np.float32)
    i = np.asarray(distance_membership_i, np.int64)
    j = np.asarray(distance_membership_j, np.int64)
    dist_bf = np.asarray(distance, np.float32).astype(NPBF)
    af_bf = af.astype(NPBF)
    af_ext = np.concatenate([af_bf, np.zeros((1, P), NPBF)], axis=0)
    counts = np.bincount(i, minlength=N_ATOMS)

    wdfe = np.zeros((DPAD, P), np.float32)
    wdfe[:100] = np.asarray(W_df, np.float32)
    wdfe[100] = np.asarray(b_df, np.float32)
    cp16 = np.zeros((P, C16W), np.float32)
    cp16[:, 0:P] = np.asarray(W_cf, np.float32)
    cp16[:, P:2 * P] = np.asarray(W_fc, np.float32)
    cp16[:DPAD, 2 * P:3 * P] = wdfe
    cp16[:, 3 * P:4 * P] = np.arange(P, dtype=np.float32)[None, :]
    cp16[:, 4 * P:5 * P] = np.eye(P, dtype=np.float32)
    cp16[0, 5 * P:5 * P + P] = np.asarray(b_cf, np.float32)
    cp16[0, 5 * P + P:5 * P + 640] = 1.0
    cp32 = np.zeros((P, 2), np.float32)
    cp32[:, 0] = np.asarray(b_cf, np.float32)
    cp32[:, 1] = np.asarray(b_df, np.float32)
    shared = {"cp16": cp16.astype(NPBF), "cp32": cp32}

    in_maps = []
    outmaps = []
    for c in range(NCORES):
        a_lo, a_hi = c * APC, (c + 1) * APC
        cnt = counts[a_lo:a_hi]
        # greedy max-fill: window takes atoms while <=128 atoms & <=CAP pairs
        bounds = [0]
        pos = 0
        while pos < APC:
            take, s = 0, 0
            while take < P and pos + take < APC and \
                    s + cnt[pos + take] <= CAP:
                s += cnt[pos + take]
                take += 1
            assert take > 0, "single atom exceeds window capacity"
            pos += take
            bounds.append(pos)
        assert len(bounds) - 1 <= NWIN, f"needs {len(bounds)-1} windows"
        while len(bounds) < NWIN + 1:
            bounds.append(APC)
        bounds = np.asarray(bounds, np.int64) + a_lo
        pb = np.searchsorted(i, bounds)
        npair = pb[1:] - pb[:-1]
        natom = bounds[1:] - bounds[:-1]
        assert npair.max() <= CAP

        colmap = np.full((NWIN, CAP), -1, np.int64)
        jmap = np.full((NWIN, GW), N_ATOMS, np.int64)
        ipr = np.full((NWIN, CAP), -1.0, np.float32)
        for w in range(NWIN):
            n = int(npair[w])
            colmap[w, :n] = np.arange(pb[w], pb[w + 1])
            jmap[w, :n] = j[pb[w]:pb[w + 1]]
            jmap[w, CAP:CAP + natom[w]] = np.arange(bounds[w], bounds[w + 1])
            ipr[w, :n] = (i[pb[w]:pb[w + 1]] - bounds[w]).astype(np.float32)

        flat = colmap.reshape(-1)
        m = flat >= 0
        dT = np.zeros((NWIN * CAP, DPAD), NPBF)
        dT[m, :100] = dist_bf[flat[m]]
        dT[m, 100] = 1.0
        distT_c = np.ascontiguousarray(dT.T)

        gaT_c = np.ascontiguousarray(af_ext[jmap.reshape(-1)].T)

        # one-hot segment-select matrices, streamed: S[p_pair, f_atom]
        Sf = np.zeros((NWIN, TPW, P, P), NPBF)
        ipr3 = ipr.reshape(NWIN, TPW, P)
        wi, si, pi = np.nonzero(ipr3 >= 0)
        Sf[wi, si, pi, ipr3[wi, si, pi].astype(np.int64)] = 1.0
        S_c = np.ascontiguousarray(
            Sf.transpose(2, 0, 1, 3).reshape(P, NWIN * CAP))

        rowmap = np.full((NWIN, P), -1, np.int64)
        outmap = np.empty(APC, np.int64)
        for w in range(NWIN):
            na = int(natom[w])
            rowmap[w, :na] = np.arange(bounds[w], bounds[w + 1])
            outmap[bounds[w] - a_lo:bounds[w + 1] - a_lo] = \
                w * P + np.arange(na)
        rflat = rowmap.reshape(-1)
        rm = rflat >= 0
        afo_c = np.zeros((NWIN * P, P), NPBF)
        afo_c[rm] = af_bf[rflat[rm]]

        mdict = {
            "distT": distT_c,
            "gaT": gaT_c,
            "Sst": S_c,
            "afo": afo_c,
        }
        mdict.update(shared)
        in_maps.append(mdict)
        outmaps.append(outmap)
    return in_maps, outmaps


def unshard(results, outmaps):
    out = np.empty((N_ATOMS, N_EMB), np.float32)
    for c in range(NCORES):
        out[c * APC:(c + 1) * APC] = results[c]["out"][outmaps[c]]
    return out


_NC_CACHE = {}


def get_nc():
    if "nc" not in _NC_CACHE:
        _NC_CACHE["nc"] = build_nc()
    return _NC_CACHE["nc"]


def kernel(**inputs):
    in_maps, outmaps = host_prep(**inputs)
    nc = get_nc()
    res = run_bass_kernel_spmd(nc, in_maps, core_ids=list(range(NCORES)))
    return unshard(res.results, outmaps)


# revision 27
# speedup vs baseline: 4.3985x; 1.0004x over previous
"""DTNNStep graph-message-passing kernel for 8x Trainium2 NeuronCores (v3).

Strategy: distance_membership_i is sorted, so pairs are sharded by
destination-atom range (6250 atoms per core). Each core processes 50
variable-width "windows" (<=128 atoms each, chosen on host so every
window holds <= 2048 pairs), so each window is exactly TPW=16 pair
tiles of 128 and the instruction stream is identical across cores.

The per-pair gather afh[j] is restructured: the HOST pre-gathers raw
atom_features[j] per pair (pure data layout) into a sequential bf16
stream gaT [128 emb, pairs]; the device computes
afh^T = W_cf^T @ gaT + b_cf per 512-pair block with a constant
stationary operand. This removes the device-side indirect-DMA gather,
the afh table build phase, and the per-tile PE transpose of the
baseline. dist^T is padded to 112 partitions (divisible by 16) so its
DMA stream sprays across all 16 DMA engines.

Pipeline per 512-pair block (all bf16 matmuls, f32 PSUM):
  dh^T  = wdfe^T @ distT-block          (PE, stationary wdfe const)
  afh^T = W_cf^T @ gaT-block            (PE, stationary W_cf const)
  biased = afh^T + b_cf                 (DVE, per-partition scalar)
  fusedT = biased * dh^T                (DVE)
  S[p,f] = (iota[f] == i'[p]) per tile  (DVE, one-hot dest-atom)
  mp     = fusedT-tile^T @ W_fc         (PE, per tile)
  msgs   = tanh(mp)                     (Scalar)
  win   += S^T @ msgs                   (PE PSUM accumulate, per tile)
Window flush computes the self-interaction term from the own-atom
columns appended to the gaT stream and writes out = win - ii + af.
No collectives: each core owns a disjoint output slice.
"""

import sys

for _p in ("/opt/trn_rl_repo",):
    if _p not in sys.path:
        sys.path.insert(0, _p)

import numpy as np
import ml_dtypes
import concourse.bass as bass
import concourse.bacc as bacc
import concourse.tile as tile
from concourse import mybir
from concourse.bass_utils import run_bass_kernel_spmd

F32 = mybir.dt.float32
BF16 = mybir.dt.bfloat16
F8E4 = mybir.dt.float8e4
NPBF = ml_dtypes.bfloat16
NPF8 = ml_dtypes.float8_e4m3

P = 128
N_ATOMS = 50000
N_PAIRS = 800000
N_EMB = 128
NCORES = 8
APC = N_ATOMS // NCORES            # atoms per core: 6250
TPW = 16                           # pair tiles per window
CAP = TPW * P                      # pair capacity per window: 2048
NWIN = 50                          # windows per core
NBLK = TPW // 4                    # 4-tile (512-pair) blocks per window
DPAD = 112                         # dist rows: 100 + bias row + pad (16|112)
GW = CAP + P                       # gaT cols per window (pairs + own atoms)
C16W = 5 * P + 640                 # bf16 const pack width


def build_nc():
    nc = bacc.Bacc()

    distT = nc.declare_dram_parameter("distT", [DPAD, NWIN * CAP], BF16,
                                      isOutput=False)
    gaT_d = nc.declare_dram_parameter("gaT", [P, NWIN * GW], BF16,
                                      isOutput=False)
    S_d = nc.declare_dram_parameter("Sst", [P, NWIN * CAP], F8E4,
                                    isOutput=False)
    cp16_d = nc.declare_dram_parameter("cp16", [P, C16W], BF16, isOutput=False)
    cp32_d = nc.declare_dram_parameter("cp32", [P, 2], F32, isOutput=False)
    out_d = nc.declare_dram_parameter("out", [NWIN * P, P], F32, isOutput=True)

    AT = mybir.AluOpType
    Tanh = mybir.ActivationFunctionType.Tanh
    Ident = mybir.ActivationFunctionType.Identity

    with tile.TileContext(nc) as tc:
        with (
            tc.tile_pool(name="consts", bufs=1) as cpool,
            tc.tile_pool(name="dist", bufs=3) as dist_pool,
            tc.tile_pool(name="ga", bufs=3) as ga_pool,
            tc.tile_pool(name="biased", bufs=3) as b_pool,
            tc.tile_pool(name="fusedT", bufs=3) as f_pool,
            tc.tile_pool(name="sgen", bufs=2) as s_pool,
            tc.tile_pool(name="msgs", bufs=3) as m_pool,
            tc.tile_pool(name="flush", bufs=2) as fl_pool,
            tc.tile_pool(name="ps_dh", bufs=2, space="PSUM") as dh_ps,
            tc.tile_pool(name="ps_ah", bufs=2, space="PSUM") as ah_ps,
            tc.tile_pool(name="ps_m", bufs=2, space="PSUM") as m_ps,
            tc.tile_pool(name="ps_win", bufs=2, space="PSUM") as win_ps,
        ):
            fl_ps = m_ps
            cpk = cpool.tile([P, C16W], BF16)
            nc.sync.dma_start(cpk[:], cp16_d[:])
            wcf = cpk[:, 0:P]
            wfc = cpk[:, P:2 * P]
            wdfe = cpk[:DPAD, 2 * P:3 * P]
            iota = cpk[:, 3 * P:4 * P]
            ident = cpk[:, 4 * P:5 * P]
            bcfr = cpk[0:1, 5 * P:5 * P + P]
            ones = cpk[0:1, 5 * P + P:5 * P + 640]
            cpk32 = cpool.tile([P, 2], F32)
            nc.sync.dma_start(cpk32[:], cp32_d[:])
            bcf = cpk32[:, 0:1]
            bdf = cpk32[:, 1:2]
            for w in range(NWIN):
                dt = dist_pool.tile([DPAD, CAP], BF16)
                nc.sync.dma_start(dt[:], distT[:, w * CAP:(w + 1) * CAP])
                ga = ga_pool.tile([P, GW], BF16)
                nc.scalar.dma_start(ga[:], gaT_d[:, w * GW:(w + 1) * GW])
                St = s_pool.tile([P, CAP], F8E4)
                nc.gpsimd.dma_start(St[:], S_d[:, w * CAP:(w + 1) * CAP])

                win = win_ps.tile([P, P], F32)
                # residual init: win = (ga_own)^T @ I = af rows of window
                nc.tensor.matmul(win[:], lhsT=ga[:, CAP:CAP + P], rhs=ident,
                                 start=True, stop=False)

                # prologue: first block's matmuls + flush afh so PE can
                # run ahead of the DVE/Scalar stages
                ah0 = ah_ps.tile([P, 512], F32, tag="ah")
                nc.tensor.matmul(ah0[:], lhsT=wcf, rhs=ga[:, 0:512],
                                 start=True, stop=True)
                dh0 = dh_ps.tile([P, 512], F32, tag="dh")
                nc.tensor.matmul(dh0[:], lhsT=wdfe, rhs=dt[:, 0:512],
                                 start=True, stop=True)
                aho = fl_ps.tile([P, P], F32, tag="mp")
                nc.tensor.matmul(aho[:], lhsT=wcf, rhs=ga[:, CAP:CAP + P],
                                 start=True, stop=True)
                ipre = fl_pool.tile([P, P], BF16, tag="ipre")
                nc.vector.tensor_scalar(out=ipre[:], in0=aho[:], scalar1=bcf,
                                        scalar2=bdf, op0=AT.add, op1=AT.mult)

                ah, dh = ah0, dh0
                for b in range(NBLK):
                    # issue next block's heavy matmuls first (pipelining)
                    if b + 1 < NBLK:
                        c1 = (b + 1) * 512
                        ahn = ah_ps.tile([P, 512], F32, tag="ah")
                        nc.tensor.matmul(ahn[:], lhsT=wcf,
                                         rhs=ga[:, c1:c1 + 512],
                                         start=True, stop=True)
                        dhn = dh_ps.tile([P, 512], F32, tag="dh")
                        nc.tensor.matmul(dhn[:], lhsT=wdfe,
                                         rhs=dt[:, c1:c1 + 512],
                                         start=True, stop=True)
                    biased = b_pool.tile([P, 512], BF16)
                    if b % 2 == 0:
                        nc.scalar.activation(biased[:], ah[:], Ident,
                                             bias=bcf)
                    else:
                        nc.vector.tensor_scalar(out=biased[:], in0=ah[:],
                                                scalar1=bcf, scalar2=None,
                                                op0=AT.add)
                    fusedT = f_pool.tile([P, 512], BF16)
                    nc.vector.tensor_tensor(fusedT[:], biased[:], dh[:],
                                            op=AT.mult)
                    mp = m_ps.tile([P, 512], F32, tag="mp")
                    for s in range(4):
                        nc.tensor.matmul(mp[:, s * P:(s + 1) * P],
                                         lhsT=fusedT[:, s * P:(s + 1) * P],
                                         rhs=wfc, start=True, stop=True)
                    msgs = m_pool.tile([P, 512], BF16)
                    nc.scalar.activation(msgs[:], mp[:], Tanh)
                    for s in range(4):
                        kk = b * 4 + s
                        nc.tensor.matmul(win[:],
                                         lhsT=St[:, kk * P:(kk + 1) * P],
                                         rhs=msgs[:, s * P:(s + 1) * P],
                                         start=False, stop=(kk == TPW - 1))
                    if b + 1 < NBLK:
                        ah, dh = ahn, dhn

                # ---- window flush ----
                iips = fl_ps.tile([P, P], F32, tag="mp")
                nc.tensor.matmul(iips[:], lhsT=ipre[:], rhs=wfc,
                                 start=True, stop=True)
                ii = fl_pool.tile([P, P], F32, tag="ii")
                nc.scalar.activation(ii[:], iips[:], Tanh)
                res = fl_pool.tile([P, P], F32, tag="res")
                nc.vector.tensor_tensor(res[:], win[:], ii[:],
                                        op=AT.subtract)
                nc.sync.dma_start(out_d[w * P:(w + 1) * P, :], res[:])

    nc.compile()
    return nc


def host_prep(atom_features, distance, atom_membership,
              distance_membership_i, distance_membership_j,
              W_cf, W_df, W_fc, b_cf, b_df):
    """Pack per-core inputs. Returns (in_maps, outmaps) where outmaps[c]
    maps each core-local atom row to its padded out-tensor row."""
    af = np.asarray(atom_features, np.float32)
    i = np.asarray(distance_membership_i, np.int64)
    j = np.asarray(distance_membership_j, np.int64)
    dist_bf = np.asarray(distance, np.float32).astype(NPBF)
    af_bf = af.astype(NPBF)
    af_ext = np.concatenate([af_bf, np.zeros((1, P), NPBF)], axis=0)
    counts = np.bincount(i, minlength=N_ATOMS)

    wdfe = np.zeros((DPAD, P), np.float32)
    wdfe[:100] = np.asarray(W_df, np.float32)
    wdfe[100] = np.asarray(b_df, np.float32)
    cp16 = np.zeros((P, C16W), np.float32)
    cp16[:, 0:P] = np.asarray(W_cf, np.float32)
    cp16[:, P:2 * P] = np.asarray(W_fc, np.float32)
    cp16[:DPAD, 2 * P:3 * P] = wdfe
    cp16[:, 3 * P:4 * P] = np.arange(P, dtype=np.float32)[None, :]
    cp16[:, 4 * P:5 * P] = np.eye(P, dtype=np.float32)
    cp16[0, 5 * P:5 * P + P] = np.asarray(b_cf, np.float32)
    cp16[0, 5 * P + P:5 * P + 640] = 1.0
    cp32 = np.zeros((P, 2), np.float32)
    cp32[:, 0] = np.asarray(b_cf, np.float32)
    cp32[:, 1] = np.asarray(b_df, np.float32)
    shared = {"cp16": cp16.astype(NPBF), "cp32": cp32}

    in_maps = []
    outmaps = []
    for c in range(NCORES):
        a_lo, a_hi = c * APC, (c + 1) * APC
        cnt = counts[a_lo:a_hi]
        # greedy max-fill: window takes atoms while <=128 atoms & <=CAP pairs
        bounds = [0]
        pos = 0
        while pos < APC:
            take, s = 0, 0
            while take < P and pos + take < APC and \
                    s + cnt[pos + take] <= CAP:
                s += cnt[pos + take]
                take += 1
            assert take > 0, "single atom exceeds window capacity"
            pos += take
            bounds.append(pos)
        assert len(bounds) - 1 <= NWIN, f"needs {len(bounds)-1} windows"
        while len(bounds) < NWIN + 1:
            bounds.append(APC)
        bounds = np.asarray(bounds, np.int64) + a_lo
        pb = np.searchsorted(i, bounds)
        npair = pb[1:] - pb[:-1]
        natom = bounds[1:] - bounds[:-1]
        assert npair.max() <= CAP

        colmap = np.full((NWIN, CAP), -1, np.int64)
        jmap = np.full((NWIN, GW), N_ATOMS, np.int64)
        ipr = np.full((NWIN, CAP), -1.0, np.float32)
        for w in range(NWIN):
            n = int(npair[w])
            colmap[w, :n] = np.arange(pb[w], pb[w + 1])
            jmap[w, :n] = j[pb[w]:pb[w + 1]]
            jmap[w, CAP:CAP + natom[w]] = np.arange(bounds[w], bounds[w + 1])
            ipr[w, :n] = (i[pb[w]:pb[w + 1]] - bounds[w]).astype(np.float32)

        flat = colmap.reshape(-1)
        m = flat >= 0
        dT = np.zeros((NWIN * CAP, DPAD), NPBF)
        dT[m, :100] = dist_bf[flat[m]]
        dT[m, 100] = 1.0
        distT_c = np.ascontiguousarray(dT.T)

        gaT_c = np.ascontiguousarray(af_ext[jmap.reshape(-1)].T)

        # one-hot segment-select matrices, streamed: S[p_pair, f_atom]
        Sf = np.zeros((NWIN, TPW, P, P), NPF8)
        ipr3 = ipr.reshape(NWIN, TPW, P)
        wi, si, pi = np.nonzero(ipr3 >= 0)
        Sf[wi, si, pi, ipr3[wi, si, pi].astype(np.int64)] = 1.0
        S_c = np.ascontiguousarray(
            Sf.transpose(2, 0, 1, 3).reshape(P, NWIN * CAP))

        rowmap = np.full((NWIN, P), -1, np.int64)
        outmap = np.empty(APC, np.int64)
        for w in range(NWIN):
            na = int(natom[w])
            rowmap[w, :na] = np.arange(bounds[w], bounds[w + 1])
            outmap[bounds[w] - a_lo:bounds[w + 1] - a_lo] = \
                w * P + np.arange(na)
        mdict = {
            "distT": distT_c,
            "gaT": gaT_c,
            "Sst": S_c,
        }
        mdict.update(shared)
        in_maps.append(mdict)
        outmaps.append(outmap)
    return in_maps, outmaps


def unshard(results, outmaps):
    out = np.empty((N_ATOMS, N_EMB), np.float32)
    for c in range(NCORES):
        out[c * APC:(c + 1) * APC] = results[c]["out"][outmaps[c]]
    return out


_NC_CACHE = {}


def get_nc():
    if "nc" not in _NC_CACHE:
        _NC_CACHE["nc"] = build_nc()
    return _NC_CACHE["nc"]


def kernel(**inputs):
    in_maps, outmaps = host_prep(**inputs)
    nc = get_nc()
    res = run_bass_kernel_spmd(nc, in_maps, core_ids=list(range(NCORES)))
    return unshard(res.results, outmaps)


# revision 28
# speedup vs baseline: 5.2275x; 1.1885x over previous
"""DTNNStep graph-message-passing kernel for 8x Trainium2 NeuronCores (v3).

Strategy: distance_membership_i is sorted, so pairs are sharded by
destination-atom range (6250 atoms per core). Each core processes 50
variable-width "windows" (<=128 atoms each, chosen on host so every
window holds <= 2048 pairs), so each window is exactly TPW=16 pair
tiles of 128 and the instruction stream is identical across cores.

The per-pair gather afh[j] is restructured: the HOST pre-gathers raw
atom_features[j] per pair (pure data layout) into a sequential bf16
stream gaT [128 emb, pairs]; the device computes
afh^T = W_cf^T @ gaT + b_cf per 512-pair block with a constant
stationary operand. This removes the device-side indirect-DMA gather,
the afh table build phase, and the per-tile PE transpose of the
baseline. dist^T is padded to 112 partitions (divisible by 16) so its
DMA stream sprays across all 16 DMA engines.

Pipeline per 512-pair block (all bf16 matmuls, f32 PSUM):
  dh^T  = wdfe^T @ distT-block          (PE, stationary wdfe const)
  afh^T = W_cf^T @ gaT-block            (PE, stationary W_cf const)
  biased = afh^T + b_cf                 (DVE, per-partition scalar)
  fusedT = biased * dh^T                (DVE)
  S[p,f] = (iota[f] == i'[p]) per tile  (DVE, one-hot dest-atom)
  mp     = fusedT-tile^T @ W_fc         (PE, per tile)
  msgs   = tanh(mp)                     (Scalar)
  win   += S^T @ msgs                   (PE PSUM accumulate, per tile)
Window flush computes the self-interaction term from the own-atom
columns appended to the gaT stream and writes out = win - ii + af.
No collectives: each core owns a disjoint output slice.
"""

import sys

for _p in ("/opt/trn_rl_repo",):
    if _p not in sys.path:
        sys.path.insert(0, _p)

import numpy as np
import ml_dtypes
import concourse.bass as bass
import concourse.bacc as bacc
import concourse.tile as tile
from concourse import mybir
from concourse.bass_utils import run_bass_kernel_spmd

F32 = mybir.dt.float32
BF16 = mybir.dt.bfloat16
F8E4 = mybir.dt.float8e4
NPBF = ml_dtypes.bfloat16
NPF8 = ml_dtypes.float8_e4m3

P = 128
N_ATOMS = 50000
N_PAIRS = 800000
N_EMB = 128
NCORES = 8
APC = N_ATOMS // NCORES            # atoms per core: 6250
TPW = 16                           # pair tiles per window
CAP = TPW * P                      # pair capacity per window: 2048
NWIN = 50                          # windows per core
NBLK = TPW // 4                    # 4-tile (512-pair) blocks per window
DPAD = 112                         # dist rows: 100 + bias row + pad (16|112)
GW = CAP + P                       # gaT cols per window (pairs + own atoms)
C16W = 5 * P + 640                 # bf16 const pack width


def build_nc():
    nc = bacc.Bacc()

    distT = nc.declare_dram_parameter("distT", [DPAD, NWIN * CAP], BF16,
                                      isOutput=False)
    gaT_d = nc.declare_dram_parameter("gaT", [P, NWIN * GW], BF16,
                                      isOutput=False)
    S_d = nc.declare_dram_parameter("Sst", [P, NWIN * CAP], F8E4,
                                    isOutput=False)
    cp16_d = nc.declare_dram_parameter("cp16", [P, C16W], BF16, isOutput=False)
    cp32_d = nc.declare_dram_parameter("cp32", [P, 2], F32, isOutput=False)
    out_d = nc.declare_dram_parameter("out", [NWIN * P, P], F32, isOutput=True)

    AT = mybir.AluOpType
    Tanh = mybir.ActivationFunctionType.Tanh
    Ident = mybir.ActivationFunctionType.Identity

    with tile.TileContext(nc) as tc:
        with (
            tc.tile_pool(name="consts", bufs=1) as cpool,
            tc.tile_pool(name="dist", bufs=3) as dist_pool,
            tc.tile_pool(name="ga", bufs=3) as ga_pool,
            tc.tile_pool(name="biased", bufs=3) as b_pool,
            tc.tile_pool(name="fusedT", bufs=3) as f_pool,
            tc.tile_pool(name="sgen", bufs=2) as s_pool,
            tc.tile_pool(name="msgs", bufs=3) as m_pool,
            tc.tile_pool(name="flush", bufs=2) as fl_pool,
            tc.tile_pool(name="ps_dh", bufs=2, space="PSUM") as dh_ps,
            tc.tile_pool(name="ps_ah", bufs=2, space="PSUM") as ah_ps,
            tc.tile_pool(name="ps_m", bufs=2, space="PSUM") as m_ps,
            tc.tile_pool(name="ps_win", bufs=2, space="PSUM") as win_ps,
        ):
            fl_ps = m_ps
            cpk = cpool.tile([P, C16W], BF16)
            nc.sync.dma_start(cpk[:], cp16_d[:])
            wcf = cpk[:, 0:P]
            wfc = cpk[:, P:2 * P]
            wdfe = cpk[:DPAD, 2 * P:3 * P]
            negident = cpk[:, 3 * P:4 * P]
            ident = cpk[:, 4 * P:5 * P]
            bcfr = cpk[0:1, 5 * P:5 * P + P]
            ones = cpk[0:1, 5 * P + P:5 * P + 640]
            cpk32 = cpool.tile([P, 2], F32)
            nc.sync.dma_start(cpk32[:], cp32_d[:])
            bcf = cpk32[:, 0:1]
            bdf = cpk32[:, 1:2]
            for w in range(NWIN):
                dt = dist_pool.tile([DPAD, CAP], BF16)
                nc.sync.dma_start(dt[:], distT[:, w * CAP:(w + 1) * CAP])
                ga = ga_pool.tile([P, GW], BF16)
                nc.sync.dma_start(ga[:], gaT_d[:, w * GW:(w + 1) * GW])
                St = s_pool.tile([P, CAP], F8E4)
                nc.gpsimd.dma_start(St[:], S_d[:, w * CAP:(w + 1) * CAP])

                win = win_ps.tile([P, P], F32)
                # residual init: win = (ga_own)^T @ I = af rows of window
                nc.tensor.matmul(win[:], lhsT=ga[:, CAP:CAP + P], rhs=ident,
                                 start=True, stop=False)

                # prologue: first block's matmuls + flush afh so PE can
                # run ahead of the DVE/Scalar stages
                ah0 = ah_ps.tile([P, 512], F32, tag="ah")
                nc.tensor.matmul(ah0[:], lhsT=wcf, rhs=ga[:, 0:512],
                                 start=True, stop=True)
                dh0 = dh_ps.tile([P, 512], F32, tag="dh")
                nc.tensor.matmul(dh0[:], lhsT=wdfe, rhs=dt[:, 0:512],
                                 start=True, stop=True)
                aho = fl_ps.tile([P, P], F32, tag="mp")
                nc.tensor.matmul(aho[:], lhsT=wcf, rhs=ga[:, CAP:CAP + P],
                                 start=True, stop=True)
                ipre = fl_pool.tile([P, P], BF16, tag="ipre")
                nc.vector.tensor_scalar(out=ipre[:], in0=aho[:], scalar1=bcf,
                                        scalar2=bdf, op0=AT.add, op1=AT.mult)

                ah, dh = ah0, dh0
                for b in range(NBLK):
                    # issue next block's heavy matmuls first (pipelining)
                    if b + 1 < NBLK:
                        c1 = (b + 1) * 512
                        ahn = ah_ps.tile([P, 512], F32, tag="ah")
                        nc.tensor.matmul(ahn[:], lhsT=wcf,
                                         rhs=ga[:, c1:c1 + 512],
                                         start=True, stop=True)
                        dhn = dh_ps.tile([P, 512], F32, tag="dh")
                        nc.tensor.matmul(dhn[:], lhsT=wdfe,
                                         rhs=dt[:, c1:c1 + 512],
                                         start=True, stop=True)
                    biased = b_pool.tile([P, 512], BF16)
                    if b % 2 == 0:
                        nc.scalar.activation(biased[:], ah[:], Ident,
                                             bias=bcf)
                    else:
                        nc.vector.tensor_scalar(out=biased[:], in0=ah[:],
                                                scalar1=bcf, scalar2=None,
                                                op0=AT.add)
                    fusedT = f_pool.tile([P, 512], BF16)
                    nc.vector.tensor_tensor(fusedT[:], biased[:], dh[:],
                                            op=AT.mult)
                    mp = m_ps.tile([P, 512], F32, tag="mp")
                    for s in range(4):
                        nc.tensor.matmul(mp[:, s * P:(s + 1) * P],
                                         lhsT=fusedT[:, s * P:(s + 1) * P],
                                         rhs=wfc, start=True, stop=True)
                    msgs = m_pool.tile([P, 512], BF16)
                    nc.scalar.activation(msgs[:], mp[:], Tanh)
                    for s in range(4):
                        kk = b * 4 + s
                        nc.tensor.matmul(win[:],
                                         lhsT=St[:, kk * P:(kk + 1) * P],
                                         rhs=msgs[:, s * P:(s + 1) * P],
                                         start=False, stop=False)
                    if b + 1 < NBLK:
                        ah, dh = ahn, dhn

                # ---- window flush ----
                iips = fl_ps.tile([P, P], F32, tag="mp")
                nc.tensor.matmul(iips[:], lhsT=ipre[:], rhs=wfc,
                                 start=True, stop=True)
                ii = fl_pool.tile([P, P], BF16, tag="ii")
                nc.scalar.activation(ii[:], iips[:], Tanh)
                nc.tensor.matmul(win[:], lhsT=negident, rhs=ii[:],
                                 start=False, stop=True)
                res = fl_pool.tile([P, P], F32, tag="res")
                nc.scalar.copy(res[:], win[:])
                nc.sync.dma_start(out_d[w * P:(w + 1) * P, :], res[:])

    nc.compile()
    return nc


def host_prep(atom_features, distance, atom_membership,
              distance_membership_i, distance_membership_j,
              W_cf, W_df, W_fc, b_cf, b_df):
    """Pack per-core inputs. Returns (in_maps, outmaps) where outmaps[c]
    maps each core-local atom row to its padded out-tensor row."""
    af = np.asarray(atom_features, np.float32)
    i = np.asarray(distance_membership_i, np.int64)
    j = np.asarray(distance_membership_j, np.int64)
    dist_bf = np.asarray(distance, np.float32).astype(NPBF)
    af_bf = af.astype(NPBF)
    af_ext = np.concatenate([af_bf, np.zeros((1, P), NPBF)], axis=0)
    counts = np.bincount(i, minlength=N_ATOMS)

    wdfe = np.zeros((DPAD, P), np.float32)
    wdfe[:100] = np.asarray(W_df, np.float32)
    wdfe[100] = np.asarray(b_df, np.float32)
    cp16 = np.zeros((P, C16W), np.float32)
    cp16[:, 0:P] = np.asarray(W_cf, np.float32)
    cp16[:, P:2 * P] = np.asarray(W_fc, np.float32)
    cp16[:DPAD, 2 * P:3 * P] = wdfe
    cp16[:, 3 * P:4 * P] = -np.eye(P, dtype=np.float32)
    cp16[:, 4 * P:5 * P] = np.eye(P, dtype=np.float32)
    cp16[0, 5 * P:5 * P + P] = np.asarray(b_cf, np.float32)
    cp16[0, 5 * P + P:5 * P + 640] = 1.0
    cp32 = np.zeros((P, 2), np.float32)
    cp32[:, 0] = np.asarray(b_cf, np.float32)
    cp32[:, 1] = np.asarray(b_df, np.float32)
    shared = {"cp16": cp16.astype(NPBF), "cp32": cp32}

    in_maps = []
    outmaps = []
    for c in range(NCORES):
        a_lo, a_hi = c * APC, (c + 1) * APC
        cnt = counts[a_lo:a_hi]
        # greedy max-fill: window takes atoms while <=128 atoms & <=CAP pairs
        bounds = [0]
        pos = 0
        while pos < APC:
            take, s = 0, 0
            while take < P and pos + take < APC and \
                    s + cnt[pos + take] <= CAP:
                s += cnt[pos + take]
                take += 1
            assert take > 0, "single atom exceeds window capacity"
            pos += take
            bounds.append(pos)
        assert len(bounds) - 1 <= NWIN, f"needs {len(bounds)-1} windows"
        while len(bounds) < NWIN + 1:
            bounds.append(APC)
        bounds = np.asarray(bounds, np.int64) + a_lo
        pb = np.searchsorted(i, bounds)
        npair = pb[1:] - pb[:-1]
        natom = bounds[1:] - bounds[:-1]
        assert npair.max() <= CAP

        colmap = np.full((NWIN, CAP), -1, np.int64)
        jmap = np.full((NWIN, GW), N_ATOMS, np.int64)
        ipr = np.full((NWIN, CAP), -1.0, np.float32)
        for w in range(NWIN):
            n = int(npair[w])
            colmap[w, :n] = np.arange(pb[w], pb[w + 1])
            jmap[w, :n] = j[pb[w]:pb[w + 1]]
            jmap[w, CAP:CAP + natom[w]] = np.arange(bounds[w], bounds[w + 1])
            ipr[w, :n] = (i[pb[w]:pb[w + 1]] - bounds[w]).astype(np.float32)

        flat = colmap.reshape(-1)
        m = flat >= 0
        dT = np.zeros((NWIN * CAP, DPAD), NPBF)
        dT[m, :100] = dist_bf[flat[m]]
        dT[m, 100] = 1.0
        distT_c = np.ascontiguousarray(dT.T)

        gaT_c = np.ascontiguousarray(af_ext[jmap.reshape(-1)].T)

        # one-hot segment-select matrices, streamed: S[p_pair, f_atom]
        Sf = np.zeros((NWIN, TPW, P, P), NPF8)
        ipr3 = ipr.reshape(NWIN, TPW, P)
        wi, si, pi = np.nonzero(ipr3 >= 0)
        Sf[wi, si, pi, ipr3[wi, si, pi].astype(np.int64)] = 1.0
        S_c = np.ascontiguousarray(
            Sf.transpose(2, 0, 1, 3).reshape(P, NWIN * CAP))

        rowmap = np.full((NWIN, P), -1, np.int64)
        outmap = np.empty(APC, np.int64)
        for w in range(NWIN):
            na = int(natom[w])
            rowmap[w, :na] = np.arange(bounds[w], bounds[w + 1])
            outmap[bounds[w] - a_lo:bounds[w + 1] - a_lo] = \
                w * P + np.arange(na)
        mdict = {
            "distT": distT_c,
            "gaT": gaT_c,
            "Sst": S_c,
        }
        mdict.update(shared)
        in_maps.append(mdict)
        outmaps.append(outmap)
    return in_maps, outmaps


def unshard(results, outmaps):
    out = np.empty((N_ATOMS, N_EMB), np.float32)
    for c in range(NCORES):
        out[c * APC:(c + 1) * APC] = results[c]["out"][outmaps[c]]
    return out


_NC_CACHE = {}


def get_nc():
    if "nc" not in _NC_CACHE:
        _NC_CACHE["nc"] = build_nc()
    return _NC_CACHE["nc"]


def kernel(**inputs):
    in_maps, outmaps = host_prep(**inputs)
    nc = get_nc()
    res = run_bass_kernel_spmd(nc, in_maps, core_ids=list(range(NCORES)))
    return unshard(res.results, outmaps)
